# revision 1
# baseline (speedup 1.0000x reference)
"""Trainium2 Bass kernel for segmented min/max + MLP (MinMaxDiffSetFeat).

Computation (reference):
    seg = row -> segment id from CSR pointers
    h = concat([x, x - seg_min[seg], x - seg_max[seg]], 1) @ w1 -> lrelu -> @ w2 -> lrelu

Device strategy (per core, data-parallel over segments):
  - Host splits segments across 8 cores at CSR boundaries near N/8 multiples.
  - Host sends x transposed, stacked and bf16: partitions 0-63 = x^T,
    64-127 = -x^T.  bf16 halves DMA bytes and runs the PE at full rate.
  - Segmented min via tensor_tensor_scan with reset: state = min(state + r, x)
    where r = +BIG at segment starts. Fwd scan + bwd scan (negative-stride
    APs) => per-element full-segment min. The -x half yields -seg_max for
    free in the same [128, F] scan.
  - Reset row is DMA-broadcast from DRAM to all 128 partitions in one
    descriptor-replicated transfer per chunk.
  - MLP decomposition: h@w1 = x@(A+B+C) - seg_min@B - seg_max@C  (w1=[A;B;C]),
    so per 512-slice: two accumulating matmuls (K=64 + K=128) -> PSUM.
    Activations run [128, 1024] wide (two 512-slices on partitions 0:64 and
    two on 64:128) so ACT pays its fixed cost once per 2048 elements.
  - Output is written bf16 in a packed [128, RP/2] layout; host unpacks.
"""

import os
import sys

import numpy as np

for _p in ("/opt/trn_rl_repo",):
    if _p not in sys.path and os.path.isdir(_p):
        sys.path.insert(0, _p)

N = 500_000
D = 64
M = 8            # cores
F = int(os.environ.get("KERNEL_F", "4096"))   # chunk center width
SL = 512         # matmul slice width (one PSUM bank)
GRP = 2048       # elements covered by one [128, 1024] activation group
BIG = 1e30
BIG8 = 240.0     # reset constant when rst is fp8e4m3

LAST_EXEC_NS = None
LAST_RESULTS = None

_module_cache = {}


def _make_plan(RP):
    """Chunk width sequence: small first chunk (fast pipeline start), big
    middles (halo amortization), small tail chunks (padding trim)."""
    fmid = int(os.environ.get("KERNEL_F", str(F)))
    plan = []
    rem = RP
    w = GRP
    while w < fmid and rem >= w + fmid:   # ramp 2048 -> 4096 -> ... -> fmid
        plan.append(w)
        rem -= w
        w *= 2
    while rem > fmid:   # leave a small-chunk tail: shorter serial MLP drain
        plan.append(fmid)
        rem -= fmid
    while rem > 0:
        plan.append(GRP)
        rem -= GRP
    return plan


def _build_module(RP, H):
    import concourse.mybir as mybir
    from concourse import bacc
    from concourse.tile import TileContext

    plan = _make_plan(RP)
    lim = int(os.environ.get("KERNEL_NCHUNKS", "0"))
    if lim:
        plan = plan[:lim]
    # NB: the scan opcode is NOT valid on the Pool engine (neuronxcc
    # "Instruction engine check failed (Pool)") — scans must run on DVE.
    bwd_engine = os.environ.get("KERNEL_BWD", "dve")
    rst8 = os.environ.get("KERNEL_RST8", "1") == "1"

    nc = bacc.Bacc("TRN2")
    bf16 = mybir.dt.bfloat16
    f32 = mybir.dt.float32
    rdt = mybir.dt.float8e4 if rst8 else bf16
    xs = nc.dram_tensor("xs", [128, RP], bf16, kind="ExternalInput")
    rst = nc.dram_tensor("rst", [1, RP + 1], rdt, kind="ExternalInput")
    wp = nc.dram_tensor("wp", [128, 256], bf16, kind="ExternalInput")
    yT = nc.dram_tensor("yT", [128, RP // 2], bf16, kind="ExternalOutput")

    fmin = mybir.AluOpType.min
    fadd = mybir.AluOpType.add
    # KERNEL_SIMACT=relu: CoreSim doesn't implement Prelu; substitute Relu
    # for simulator-side numeric checks (matched by a relu reference).
    if os.environ.get("KERNEL_SIMACT", "") == "relu":
        lrelu = mybir.ActivationFunctionType.Relu
    else:
        lrelu = mybir.ActivationFunctionType.Prelu
    rbig = BIG8 if rst8 else BIG

    with TileContext(nc) as tc:
        dbufs = int(os.environ.get("KERNEL_DBUFS", "2"))
        with tc.tile_pool(name="wpool", bufs=1) as wpool, \
             tc.tile_pool(name="data", bufs=dbufs) as dpool, \
             tc.tile_pool(name="mmio", bufs=3) as mpool, \
             tc.tile_pool(name="ps1", bufs=2, space="PSUM") as p1pool, \
             tc.tile_pool(name="ps2", bufs=2, space="PSUM") as p2pool:
            # wt rides the (otherwise idle) gpsimd SWDGE queue so the first
            # chunk's rs/xx DMAs get the HWDGE + SP queue to themselves.
            wt = wpool.tile([128, 256], bf16, tag="wt")
            nc.gpsimd.dma_start(out=wt[:], in_=wp[:, :])
            bias = wpool.tile([128, 1], f32, tag="bias")
            nc.vector.memset(bias[:], 0.0)
            alpha = wpool.tile([128, 1], f32, tag="alpha")
            nc.vector.memset(alpha[:], 0.2)

            # Scan opcode is DVE-only: neuronxcc rejects it on Pool
            # ("Instruction engine check failed") and on ACT
            # ("Assertion failure: validTSPonACT").
            bwd = nc.gpsimd if bwd_engine == "pool" else nc.vector

            c0 = 0
            prev_pref = None    # (tile, width) for fwd state chaining
            for ki, Fk in enumerate(plan):
                # fwd scans carry prefix state across chunks: the initial is
                # the previous chunk's interior column at absolute position
                # c0-1, so no left halo is scanned. The right halo H stays
                # inside each fwd scan (feeds this chunk's own bwd scan).
                # The final chunk needs no right halo: trailing padding is
                # all segment starts and real segments end before RP.
                FH = Fk + (0 if ki == len(plan) - 1 else H)
                rs = dpool.tile([128, FH + 1], rdt, tag="rs")
                xx = dpool.tile([128, FH], bf16, tag="xx")
                pref = dpool.tile([128, FH], bf16, tag="pref")
                init = (rbig if prev_pref is None
                        else prev_pref[0][:, prev_pref[1] - 1:prev_pref[1]])
                # chunk 0 starts the pipeline: load + scan in two halves so
                # the first scan overlaps the second half's DMA.
                splits = [(0, 1024), (1024, FH)] if ki == 0 else [(0, FH)]
                for (a, b) in splits:
                    nc.sync.dma_start(
                        out=rs[:, a:b + (1 if b == FH else 0)],
                        in_=rst[0:1, c0 + a:c0 + b + (1 if b == FH else 0)]
                            .unsqueeze(1)
                            .broadcast_to([1, 128, b - a + (1 if b == FH
                                                            else 0)]))
                    nc.sync.dma_start(out=xx[:, a:b],
                                      in_=xs[:, c0 + a:c0 + b])
                for (a, b) in splits:
                    nc.vector.tensor_tensor_scan(
                        out=pref[:, a:b], data0=rs[:, a:b], data1=xx[:, a:b],
                        initial=(init if a == 0 else pref[:, a - 1:a]),
                        op0=fadd, op1=fmin)
                prev_pref = (pref, Fk)
                # suffix-min over pref == full segment min at every element;
                # segments covering center elements end before c0 + Fk + H.
                comb = dpool.tile([128, FH], bf16, tag="comb")
                bwd.tensor_tensor_scan(
                    out=comb[:, ::-1], data0=rs[:, 1:FH + 1][:, ::-1],
                    data1=pref[:, 0:FH][:, ::-1],
                    initial=rbig, op0=fadd, op1=fmin)

                if os.environ.get("KERNEL_STAGE", "") == "scan":
                    nc.sync.dma_start(out=yT[:, c0 // 2:(c0 + Fk) // 2],
                                      in_=comb[:, 0:Fk:2])
                    c0 += Fk
                    continue

                # The final chunk's MLP sits serially after the last scan;
                # split its activations/outputs into 512-wide halves
                # (PSUM deps are bank-level) to shorten that tail chain.
                fine = ki == len(plan) - 1
                for g in range(Fk // GRP):
                    o0 = g * GRP
                    ps1 = p1pool.tile([128, 1024], f32, tag="ps1")
                    for s in (0, 2, 1, 3):
                        p0 = 64 * (s // 2)
                        cw = (s % 2) * SL
                        el = o0 + s * SL
                        nc.tensor.matmul(
                            ps1[p0:p0 + 64, cw:cw + SL],
                            wt[0:64, 0:64], xx[0:64, el:el + SL],
                            start=True, stop=False)
                        nc.tensor.matmul(
                            ps1[p0:p0 + 64, cw:cw + SL],
                            wt[0:128, 64:128], comb[0:128, el:el + SL],
                            start=False, stop=True)
                    ps2 = p2pool.tile([128, 1024], f32, tag="ps2")
                    yc = (c0 + o0) // 2
                    if fine:
                        hh = []
                        for half in range(2):
                            cw = half * SL
                            h1h = mpool.tile([128, SL], bf16,
                                             tag=f"h1f{half}")
                            nc.scalar.activation(
                                h1h[:], ps1[:, cw:cw + SL], lrelu,
                                bias[:, 0:1], alpha=alpha[:, 0:1])
                            hh.append(h1h)
                        for s in (0, 2, 1, 3):
                            p0 = 64 * (s // 2)
                            cw = (s % 2) * SL
                            nc.tensor.matmul(
                                ps2[p0:p0 + 64, cw:cw + SL],
                                wt[p0:p0 + 64, 192:256],
                                hh[s % 2][p0:p0 + 64, :],
                                start=True, stop=True)
                        for half in range(2):
                            cw = half * SL
                            yoh = mpool.tile([128, SL], bf16,
                                             tag=f"yof{half}")
                            nc.scalar.activation(
                                yoh[:], ps2[:, cw:cw + SL], lrelu,
                                bias[:, 0:1], alpha=alpha[:, 0:1])
                            nc.sync.dma_start(
                                out=yT[:, yc + cw:yc + cw + SL], in_=yoh[:])
                        continue
                    h1 = mpool.tile([128, 1024], bf16, tag="h1")
                    nc.scalar.activation(h1[:], ps1[:], lrelu,
                                         bias[:, 0:1], alpha=alpha[:, 0:1])
                    for s in range(4):
                        p0 = 64 * (s // 2)
                        cw = (s % 2) * SL
                        nc.tensor.matmul(
                            ps2[p0:p0 + 64, cw:cw + SL],
                            wt[p0:p0 + 64, 192:256],
                            h1[p0:p0 + 64, cw:cw + SL],
                            start=True, stop=True)
                    yo = mpool.tile([128, 1024], bf16, tag="yo")
                    nc.scalar.activation(yo[:], ps2[:], lrelu,
                                         bias[:, 0:1], alpha=alpha[:, 0:1])
                    nc.sync.dma_start(out=yT[:, yc:yc + 1024], in_=yo[:])
                c0 += Fk
    nc.finalize()
    return nc


def _prepare(inputs):
    from ml_dtypes import bfloat16, float8_e4m3

    x = np.ascontiguousarray(np.asarray(inputs["x"], dtype=np.float32))
    csr = np.asarray(inputs["csr_idx"]).astype(np.int64)
    w1 = np.asarray(inputs["w1"], dtype=np.float32)
    w2 = np.asarray(inputs["w2"], dtype=np.float32)
    n, d = x.shape
    assert d == D
    rst8 = os.environ.get("KERNEL_RST8", "1") == "1"
    rdt = float8_e4m3 if rst8 else bfloat16
    rbig = BIG8 if rst8 else BIG

    # --- segment-aligned core cuts near k*n/M ---
    cuts = [0]
    for kk in range(1, M):
        target = kk * n // M
        gi = int(np.searchsorted(csr, target))
        lo = csr[gi - 1] if gi > 0 else 0
        hi = csr[gi] if gi < len(csr) else n
        cuts.append(int(hi if hi - target <= target - lo else lo))
    cuts.append(n)

    Rs = [cuts[i + 1] - cuts[i] for i in range(M)]
    Rmax = max(Rs)
    RP = ((Rmax + GRP - 1) // GRP) * GRP

    seglen = np.diff(csr)
    Lmax = int(seglen.max()) if len(seglen) else 1
    # halo must cover the longest segment; round to 16 elements
    H = max(64, ((Lmax + 15) // 16) * 16)

    # reset flag at the first row of every non-empty segment
    is_start = np.zeros(n, dtype=np.float32)
    starts = csr[:-1]
    starts = starts[starts < n]
    is_start[starts] = rbig

    wpack = np.zeros((128, 256), dtype=np.float32)
    wpack[0:64, 0:64] = w1[0:64] + w1[64:128] + w1[128:192]   # Wsum
    wpack[0:64, 64:128] = -w1[64:128]                          # -B (seg_min)
    wpack[64:128, 64:128] = w1[128:192]                        # C (-seg_max)
    wpack[0:64, 192:256] = w2
    wpack[64:128, 192:256] = w2
    wpack = wpack.astype(bfloat16)

    in_maps = []
    for c in range(M):
        r0, r1 = cuts[c], cuts[c + 1]
        R = r1 - r0
        xsb = np.zeros((128, RP), dtype=bfloat16)
        xT = x[r0:r1].T.astype(bfloat16)
        xsb[0:64, 0:R] = xT
        xsb[64:128, 0:R] = -xT
        rstb = np.full((RP + 1,), rbig, dtype=np.float32)
        rstb[0:R] = is_start[r0:r1]
        in_maps.append({"xs": xsb, "rst": rstb[None, :].astype(rdt),
                        "wp": wpack})

    key = (RP, H, tuple(_make_plan(RP)))
    if key not in _module_cache:
        _module_cache[key] = _build_module(RP, H)
    nc = _module_cache[key]
    return nc, in_maps, cuts, n, RP


def kernel(**inputs):
    global LAST_EXEC_NS, LAST_RESULTS
    from concourse.bass_utils import run_bass_kernel_spmd

    nc, in_maps, cuts, n, RP = _prepare(inputs)
    trace = os.environ.get("KERNEL_TRACE", "0") == "1"
    ncores = int(os.environ.get("KERNEL_CORES", str(M)))
    try:
        res = run_bass_kernel_spmd(nc, in_maps[:ncores],
                                   core_ids=list(range(ncores)), trace=trace)
    except ModuleNotFoundError:
        if not trace:
            raise
        # NTFF profile hook unavailable in this axon build; run untraced.
        res = run_bass_kernel_spmd(nc, in_maps[:ncores],
                                   core_ids=list(range(ncores)), trace=False)
    LAST_EXEC_NS = res.exec_time_ns
    LAST_RESULTS = res

    out = np.empty((n, D), dtype=np.float32)
    ng = RP // GRP
    for c in range(len(res.results)):
        r0, r1 = cuts[c], cuts[c + 1]
        # unpack [128, RP/2] -> [RP, 64]: partition = 64*h + f,
        # col = g*1024 + j, element = g*2048 + h*1024 + j
        arr = np.asarray(res.results[c]["yT"], dtype=np.float32)
        v = arr.reshape(2, 64, ng, 1024)           # [h, f, g, j]
        v = v.transpose(2, 0, 3, 1).reshape(RP, 64)  # [g, h, j, f]
        out[r0:r1] = v[:r1 - r0]
    return out


def benchmark(n_iters=5, **inputs):
    """Time the 8-core NEFF execution with device-resident inputs.
    Returns best per-iteration seconds (incl. PJRT dispatch overhead)."""
    import time

    import jax
    from jax.sharding import Mesh, NamedSharding, PartitionSpec
    from jax.experimental.shard_map import shard_map

    import concourse.mybir as mybir
    from concourse import bass2jax

    bass2jax.install_neuronx_cc_hook()
    nc, in_maps, cuts, n, RP = _prepare(inputs)

    partition_name = (nc.partition_id_tensor.name
                      if nc.partition_id_tensor else None)
    in_names, out_names, out_avals = [], [], []
    for alloc in nc.m.functions[0].allocations:
        if not isinstance(alloc, mybir.MemoryLocationSet):
            continue
        name = alloc.memorylocations[0].name
        if alloc.kind == "ExternalInput":
            if name != partition_name:
                in_names.append(name)
        elif alloc.kind == "ExternalOutput":
            out_names.append(name)
            out_avals.append(jax.core.ShapedArray(
                tuple(alloc.tensor_shape), mybir.dt.np(alloc.dtype)))
    n_params = len(in_names)
    zero_shapes = [((M * a.shape[0], *a.shape[1:]), a.dtype)
                   for a in out_avals]
    all_names = in_names + out_names
    if partition_name is not None:
        all_names.append(partition_name)

    def _body(*args):
        operands = list(args)
        if partition_name is not None:
            operands.append(bass2jax.partition_id_tensor())
        return tuple(bass2jax._bass_exec_p.bind(
            *operands,
            out_avals=tuple(out_avals),
            in_names=tuple(all_names),
            out_names=tuple(out_names),
            lowering_input_output_aliases=(),
            sim_require_finite=True,
            sim_require_nnan=True,
            nc=nc,
        ))

    devices = jax.devices()[:M]
    mesh = Mesh(np.asarray(devices), ("core",))
    n_outs = len(out_names)
    fn = jax.jit(shard_map(
        _body, mesh=mesh,
        in_specs=(PartitionSpec("core"),) * (n_params + n_outs),
        out_specs=(PartitionSpec("core"),) * n_outs,
        check_rep=False), keep_unused=True)

    sh = NamedSharding(mesh, PartitionSpec("core"))
    dev_in = [jax.device_put(
        np.concatenate([in_maps[c][nm] for c in range(M)], axis=0), sh)
        for nm in in_names]
    dev_zero = [jax.device_put(np.zeros(zs, dt), sh)
                for zs, dt in zero_shapes]
    out = fn(*dev_in, *dev_zero)
    jax.block_until_ready(out)
    best = float("inf")
    for _ in range(n_iters):
        t0 = time.perf_counter()
        out = fn(*dev_in, *dev_zero)
        jax.block_until_ready(out)
        best = min(best, time.perf_counter() - t0)
    return best

